# revision 27
# baseline (speedup 1.0000x reference)
"""Causal self-attention (B=4, S=2048, H=1024, NH=16) on 8 Trainium2 cores.

Sharding: core c -> (batch b = c//2, head-group g = c%2). Each core computes
8 heads (512 model dims) for one batch: QKV projections, causal attention,
and a partial output projection. Host sums the two head-group partials per
batch and adds bo.

v2 layout/schedule choices (vs v1):
  - Software-pipelined: for each s-chunk of 512, emit projections (P1),
    then attention blocks for q-chunk i=sc (P2, needs k/v chunks <= sc
    only, thanks to causality), then the output projection rows (P3).
    Engines overlap across phases instead of running phase-serial.
  - qt/kt/outT/wo in bf16 (halves SBUF, full-rate matmuls); x and the
    QKV weights stay fp32r.
  - Causal mask via precomputed band-mask tiles multiplied on DVE (bf16,
    2x/4x modes) instead of per-tile gpsimd affine_select (which costs
    ~1.5us/tile on the Pool engine).
  - V bias added on DVE from a broadcast tile (built once with a K=1
    matmul) instead of one bias matmul per V tile.
  - Score psum tiles hold both heads of a pair for one sk-chunk, so one
    exp and one mask-mul cover both heads.
  - P3 psum drain on the scalar engine to keep DVE free.
"""

import os
from collections import deque
from contextlib import ExitStack

import numpy as np

import concourse.mybir as mybir
from concourse import bacc
from concourse.tile import TileContext
from concourse.bass_utils import run_bass_kernel_spmd
from concourse import bass_isa

F32 = mybir.dt.float32
F32R = mybir.dt.float32r
BF16 = mybir.dt.bfloat16

B, S, H = 4, 2048, 1024
NH, HD = 16, 64
P = 128
DH = 512          # model dims per core (8 heads)
NHP = 4           # head pairs per core
SQC = 512         # s_q chunk
NSQ = S // SQC    # 4
NSK = S // P      # 16 s_k chunks
HO = H // P       # 8 contraction chunks for projections
NDQ = DH // P     # 4 dq tiles
SCALE = 0.125     # 1/sqrt(HD)
PT_BUFS = 6


def build_kernel() -> bacc.Bacc:
    phases = os.environ.get("BASS_PHASES", "123")
    nc = bacc.Bacc("TRN2", target_bir_lowering=False, debug=False, num_devices=8)

    xT = nc.dram_tensor("xT", [H, S], F32, kind="ExternalInput").ap()
    wqT = nc.dram_tensor("wqT", [H, DH], F32, kind="ExternalInput").ap()
    wkT = nc.dram_tensor("wkT", [H, DH], F32, kind="ExternalInput").ap()
    wvT = nc.dram_tensor("wvT", [H, DH], F32, kind="ExternalInput").ap()
    woT = nc.dram_tensor("woT", [DH, H], F32, kind="ExternalInput").ap()
    bq = nc.dram_tensor("bq", [DH], F32, kind="ExternalInput").ap()
    bk = nc.dram_tensor("bk", [DH], F32, kind="ExternalInput").ap()
    bv = nc.dram_tensor("bv", [DH], F32, kind="ExternalInput").ap()
    cmat = nc.dram_tensor("cmat", [P, P], F32, kind="ExternalInput").ap()
    out = nc.dram_tensor("out", [S, H], F32, kind="ExternalOutput").ap()

    with (
        TileContext(nc) as tc,
        ExitStack() as ctx,
        nc.allow_low_precision(reason="bf16 attention; float32r elsewhere"),
    ):
        consts = ctx.enter_context(tc.tile_pool(name="consts", bufs=1))
        persist = ctx.enter_context(tc.tile_pool(name="persist", bufs=1))
        wpool = ctx.enter_context(tc.tile_pool(name="w", bufs=1))
        xpool = ctx.enter_context(tc.tile_pool(name="x", bufs=2))
        ptp = ctx.enter_context(tc.tile_pool(name="pt", bufs=PT_BUFS))
        pstp = ctx.enter_context(tc.tile_pool(name="pst", bufs=12))
        rcp = ctx.enter_context(tc.tile_pool(name="rc", bufs=3))
        stg = ctx.enter_context(tc.tile_pool(name="p3s", bufs=4))
        bcp = ctx.enter_context(tc.tile_pool(name="bc", bufs=2))
        ps_sc = ctx.enter_context(tc.tile_pool(name="ps_sc", bufs=3, space="PSUM"))
        ps_pv = ctx.enter_context(tc.tile_pool(name="ps_pv", bufs=2, space="PSUM"))

        # ---- constants ---------------------------------------------------
        # cmat rows: 0 = head-A indicator, 32 = head-B indicator, 64 = ones
        ind_sb = consts.tile([P, P], F32R)
        nc.sync.dma_start(ind_sb[:], cmat.bitcast(F32R))
        bq_sb = consts.tile([P, NDQ], F32)
        nc.sync.dma_start(bq_sb[:], bq.rearrange("(o p) -> p o", p=P))
        bk_sb = consts.tile([P, NDQ], F32)
        nc.sync.dma_start(bk_sb[:], bk.rearrange("(o p) -> p o", p=P))
        bv_sb = consts.tile([P, DH], F32R)
        nc.sync.dma_start(bv_sb[64:65, :], bv[None, :].bitcast(F32R))

        # Band masks, shared over the head dim: mask[c][p, h, f] = f >= 128c + p
        mask_sb = consts.tile([P, NHP, 2, SQC], BF16)
        nc.any.memset(mask_sb, 1.0)
        for c in range(4):
            nc.gpsimd.affine_select(
                mask_sb[:, c, :, :], mask_sb[:, c, :, :],
                pattern=[[0, 2], [1, SQC]],
                compare_op=mybir.AluOpType.is_ge,
                fill=0.0,
                base=-P * c,
                channel_multiplier=-1,
            )

        # ---- persistent tensors ------------------------------------------
        v_sb = persist.tile([P, NSK, DH], BF16)      # V (s, dv)
        qt_sb = persist.tile([P, NDQ, S], BF16)      # Q^T (dq, s)
        kt_sb = persist.tile([P, NDQ, S], BF16)      # K^T (dq, s)
        outT_sb = persist.tile([P, NDQ, S], BF16)    # normalized attn out^T
        # [P, 2*ko + mcol, :] == flat [P, NDQ, H] slice [:, ko, mcol*512:...]
        wo_bf = persist.tile([P, HO, SQC], BF16)
        bvb_sb = persist.tile([P, DH], F32R)         # bv broadcast to 128 rows

        # ---- weights ------------------------------------------------------
        # x chunk 0 first (finely split so the first Q matmul can start as
        # soon as x[o=0] and wq[o=0] land), then the weights.
        w_sb = {}
        xq_tiles = [None] * NSQ
        xq_tiles[0] = xpool.tile([P, HO, SQC], F32R, name="xq")
        xr = xT.bitcast(F32R).rearrange("(o p) s -> p o s", p=P)
        for o in range(HO):
            nc.sync.dma_start(xq_tiles[0][:, o, :], xr[:, o, 0:SQC])
        for name, wT in (("q", wqT), ("k", wkT), ("v", wvT)):
            w_sb[name] = wpool.tile([P, HO, DH], F32R, name=f"w{name}_sb")
            for o in range(HO):
                nc.sync.dma_start(
                    w_sb[name][:, o, :],
                    wT.bitcast(F32R).rearrange("(o p) d -> p o d", p=P)[:, o, :],
                )
        # wo: stage as f32 through an x-ring slot, cast to bf16
        wo_stage = xpool.tile([P, HO, SQC], F32R, name="xq")
        for ko in range(NDQ):
            for m2 in range(2):
                nc.sync.dma_start(
                    wo_stage[:, 2 * ko + m2, :],
                    woT.bitcast(F32R).rearrange("(o p) m -> p o m", p=P)[
                        :, ko, m2 * SQC : (m2 + 1) * SQC
                    ],
                )
        nc.vector.tensor_copy(wo_bf[:], wo_stage[:])

        # bv broadcast tile: ones-row x bv row
        ps = ps_sc.tile([P, 2, SQC], F32, name="sc_ps")
        nc.tensor.matmul(
            ps[:, 0, :], ind_sb[64:65, :], bv_sb[64:65, :],
            start=True, stop=True, tile_position=(64, 0),
        )
        nc.vector.tensor_copy(bvb_sb[:], ps[:, 0, :])

        # pre-zero the pt ring so suffix-exp prefixes are finite
        for _ in range(PT_BUFS):
            ptz = ptp.tile([P, 2, SQC], BF16, name="pt")
            nc.gpsimd.memset(ptz[:], 0.0)

        def emit_p1(sc):
            ss = slice(sc * SQC, (sc + 1) * SQC)
            xq = xq_tiles[sc]
            for name, bias_sb, scale, tgt in (
                ("q", bq_sb, SCALE, qt_sb),
                ("k", bk_sb, 1.0, kt_sb),
            ):
                for tp in range(NDQ // 2):
                    pps = ps_sc.tile([P, 2, SQC], F32, name="sc_ps")
                    for td in range(2):
                        t = 2 * tp + td
                        for o in range(HO):
                            nc.tensor.matmul(
                                pps[:, td, :],
                                w_sb[name][:, o, t * P : (t + 1) * P],
                                xq[:, o, :],
                                start=(o == 0), stop=(o == HO - 1),
                            )
                        nc.vector.tensor_scalar(
                            tgt[:, t, ss], pps[:, td, :],
                            scale, bias_sb[:, t : t + 1],
                            op0=mybir.AluOpType.mult, op1=mybir.AluOpType.add,
                        )
            for vp in range(2):
                pps = ps_sc.tile([P, 2, SQC], F32, name="sc_ps")
                for vd in range(2):
                    stq = 2 * vp + vd
                    st_i = sc * (SQC // P) + stq
                    for o in range(HO):
                        nc.tensor.matmul(
                            pps[:, vd, :],
                            xq[:, o, stq * P : (stq + 1) * P],
                            w_sb["v"][:, o, :],
                            start=(o == 0), stop=(o == HO - 1),
                        )
                    nc.vector.tensor_add(v_sb[:, st_i, :], pps[:, vd, :], bvb_sb[:])

        def emit_block(hp, i):
            """Attention for head-pair hp, q-chunk i (512 rows)."""
            nj, nm = 4 * i + 4, 2 * i + 2
            sq = slice(i * SQC, (i + 1) * SQC)
            pv_ps = ps_pv.tile([P, SQC], F32, name="pv_ps")

            def emit_pv(ent):
                pts, m = ent
                for d in range(2):
                    j = 2 * m + d
                    for h in range(2):
                        dv = slice(hp * P + h * 64, hp * P + h * 64 + 64)
                        nc.tensor.matmul(
                            pv_ps[h * 64 : h * 64 + 64, :],
                            v_sb[:, j, dv],
                            pts[d][:, h, :],
                            start=(j == 0), stop=(j == nj - 1),
                            tile_position=(0, h * 64),
                        )

            pend = deque()
            # per-head pairwise reduction trees over the pair sums; each
            # level is a bf16 DVE add, the final [128,512] tile feeds one
            # ones-matmul per head for the partition sum.
            trees = [[], []]

            def tree_push(h, tile, lvl=0):
                tl = trees[h]
                while len(tl) <= lvl:
                    tl.append(None)
                if tl[lvl] is None:
                    tl[lvl] = tile
                    return
                acc = pstp.tile([P, SQC], BF16, name="pst")
                nc.vector.tensor_add(acc[:], tl[lvl][:], tile[:])
                tl[lvl] = None
                tree_push(h, acc, lvl + 1)

            def tree_finish(h):
                acc = None
                for tile in trees[h]:
                    if tile is None:
                        continue
                    if acc is None:
                        acc = tile
                        continue
                    nxt = pstp.tile([P, SQC], BF16, name="pst")
                    nc.vector.tensor_add(nxt[:], acc[:], tile[:])
                    acc = nxt
                return acc

            for m in range(nm):
                pts = []
                for d in range(2):
                    j = 2 * m + d
                    sk = slice(j * P, (j + 1) * P)
                    sc_ps = ps_sc.tile([P, 2, SQC], F32, name="sc_ps")
                    for h in range(2):
                        pb = 64 * h
                        nc.tensor.matmul(
                            sc_ps[:, h, :],
                            kt_sb[pb : pb + 64, hp, sk],
                            qt_sb[pb : pb + 64, hp, sq],
                            start=True, stop=True,
                            tile_position=(pb, 0),
                        )
                    pt = ptp.tile([P, 2, SQC], BF16, name="pt")
                    c = j - 4 * i
                    off = max(0, P * c)
                    nc.scalar.activation(
                        pt[:, :, off:], sc_ps[:, :, off:],
                        mybir.ActivationFunctionType.Exp,
                    )
                    if c >= 0:  # diagonal chunk: causal band mask
                        # full-width: zeroes the unwritten prefix too
                        nc.vector.tensor_mul(pt[:], pt[:], mask_sb[:, c, :, :])
                    pts.append(pt)
                for h in range(2):
                    pst_t = pstp.tile([P, SQC], BF16, name="pst")
                    nc.vector.tensor_add(
                        pst_t[:], pts[0][:, h, :], pts[1][:, h, :]
                    )
                    tree_push(h, pst_t)
                pend.append((pts, m))
                if len(pend) >= 2:
                    emit_pv(pend.popleft())
            while pend:
                emit_pv(pend.popleft())

            bc_sb = bcp.tile([P, SQC], F32R, name="bc_sb")
            for h, rows in ((0, slice(0, 64)), (1, slice(64, 128))):
                dsum = tree_finish(h)
                dar = rcp.tile([P, SQC], F32, name="dar")
                nc.gpsimd.partition_all_reduce(
                    dar[:], dsum[:], channels=P,
                    reduce_op=bass_isa.ReduceOp.add,
                )
                nc.vector.reciprocal(bc_sb[rows, :], dar[rows, :])
            nc.vector.tensor_mul(outT_sb[:, hp, sq], pv_ps[:], bc_sb[:])

        def emit_p3(sc):
            for st_i in range(SQC // P):
                s0 = sc * SQC + st_i * P
                ss = slice(s0, s0 + P)
                pps = ps_sc.tile([P, 2, SQC], F32, name="sc_ps")
                for mcol in range(2):
                    for ko in range(NDQ):
                        nc.tensor.matmul(
                            pps[:, mcol, :],
                            outT_sb[:, ko, ss],
                            wo_bf[:, 2 * ko + mcol, :],
                            start=(ko == 0), stop=(ko == NDQ - 1),
                        )
                for mcol in range(2):
                    ms = slice(mcol * SQC, (mcol + 1) * SQC)
                    ot = stg.tile([P, SQC], F32, name="o_stage")
                    nc.scalar.copy(ot[:], pps[:, mcol, :])
                    nc.sync.dma_start(out[ss, ms], ot[:])

        for sc in range(NSQ):
            if "1" in phases:
                emit_p1(sc)
            if sc + 1 < NSQ:  # prefetch next x chunk; overlaps with P2/P3
                xq_tiles[sc + 1] = xpool.tile([P, HO, SQC], F32R, name="xq")
                nc.sync.dma_start(
                    xq_tiles[sc + 1][:],
                    xT.bitcast(F32R).rearrange("(o p) s -> p o s", p=P)[
                        :, :, (sc + 1) * SQC : (sc + 2) * SQC
                    ],
                )
            if sc > 0 and "3" in phases:
                emit_p3(sc - 1)
            if "2" in phases:
                for hp in range(NHP):
                    emit_block(hp, sc)
        if "3" in phases:
            emit_p3(NSQ - 1)

    nc.compile()
    return nc


_NC_CACHE = [None]
LAST_RESULT = [None]


def make_in_maps(inputs):
    """Per-core input maps from the full-problem input dict."""
    x, Wq, bq, Wk, bk, Wv, bv, Wo = (
        np.asarray(inputs[k], dtype=np.float32)
        for k in ("x", "Wq", "bq", "Wk", "bk", "Wv", "bv", "Wo")
    )
    cmat = np.zeros((P, P), np.float32)
    cmat[0, 0:64] = 1.0    # head-A indicator
    cmat[32, 64:128] = 1.0  # head-B indicator
    cmat[64, :] = 1.0       # ones row (bias broadcast)
    in_maps = []
    for c in range(8):
        b, g = c // 2, c % 2
        hs = slice(DH * g, DH * (g + 1))
        in_maps.append({
            "xT": np.ascontiguousarray(x[b].T),
            "wqT": np.ascontiguousarray(Wq[hs].T),
            "wkT": np.ascontiguousarray(Wk[hs].T),
            "wvT": np.ascontiguousarray(Wv[hs].T),
            "woT": np.ascontiguousarray(Wo[:, hs].T),
            "bq": np.ascontiguousarray(bq[hs]) * np.float32(SCALE),
            "bk": np.ascontiguousarray(bk[hs]),
            "bv": np.ascontiguousarray(bv[hs]),
            "cmat": cmat,
        })
    return in_maps


def kernel(x, Wq, bq, Wk, bk, Wv, bv, Wo, bo):
    if _NC_CACHE[0] is None:
        _NC_CACHE[0] = build_kernel()
    nc = _NC_CACHE[0]

    in_maps = make_in_maps(dict(
        x=x, Wq=Wq, bq=bq, Wk=Wk, bk=bk, Wv=Wv, bv=bv, Wo=Wo,
    ))
    trace = bool(os.environ.get("BASS_PROFILE"))
    res = run_bass_kernel_spmd(
        nc, in_maps, core_ids=list(range(8)), trace=trace,
        tmpdir=os.environ.get("BASS_PROFILE_DIR") or None,
    )
    LAST_RESULT[0] = res
    bo = np.asarray(bo, dtype=np.float32)
    out = np.empty((B, S, H), np.float32)
    for b in range(B):
        out[b] = res.results[2 * b]["out"] + res.results[2 * b + 1]["out"] + bo
    return out


# revision 33
# speedup vs baseline: 2.2381x; 2.2381x over previous
"""Causal self-attention (B=4, S=2048, H=1024, NH=16) on 8 Trainium2 cores.

Sharding: core c -> (batch b = c//2, head-group g = c%2). Each core computes
8 heads (512 model dims) for one batch: QKV projections, causal attention,
and a partial output projection. Host sums the two head-group partials per
batch and adds bo.

v2 layout/schedule choices (vs v1):
  - Software-pipelined: for each s-chunk of 512, emit projections (P1),
    then attention blocks for q-chunk i=sc (P2, needs k/v chunks <= sc
    only, thanks to causality), then the output projection rows (P3).
    Engines overlap across phases instead of running phase-serial.
  - qt/kt/outT/wo in bf16 (halves SBUF, full-rate matmuls); x and the
    QKV weights stay fp32r.
  - Causal mask via precomputed band-mask tiles multiplied on DVE (bf16,
    2x/4x modes) instead of per-tile gpsimd affine_select (which costs
    ~1.5us/tile on the Pool engine).
  - V bias added on DVE from a broadcast tile (built once with a K=1
    matmul) instead of one bias matmul per V tile.
  - Score psum tiles hold both heads of a pair for one sk-chunk, so one
    exp and one mask-mul cover both heads.
  - P3 psum drain on the scalar engine to keep DVE free.
"""

import os
from collections import deque
from contextlib import ExitStack

import ml_dtypes
import numpy as np

import concourse.mybir as mybir
from concourse import bacc
from concourse.tile import TileContext
from concourse.bass_utils import run_bass_kernel_spmd
from concourse import bass_isa

F32 = mybir.dt.float32
F32R = mybir.dt.float32r
BF16 = mybir.dt.bfloat16

B, S, H = 4, 2048, 1024
NH, HD = 16, 64
P = 128
DH = 512          # model dims per core (8 heads)
NHP = 4           # head pairs per core
SQC = 512         # s_q chunk
NSQ = S // SQC    # 4
NSK = S // P      # 16 s_k chunks
HO = H // P       # 8 contraction chunks for projections
NDQ = DH // P     # 4 dq tiles
SCALE = 0.125     # 1/sqrt(HD)
PT_BUFS = 10


def build_kernel() -> bacc.Bacc:
    phases = os.environ.get("BASS_PHASES", "123")
    nc = bacc.Bacc("TRN2", target_bir_lowering=False, debug=False, num_devices=8)

    xT = nc.dram_tensor("xT", [H, S], BF16, kind="ExternalInput").ap()
    wqT = nc.dram_tensor("wqT", [H, DH], BF16, kind="ExternalInput").ap()
    wkT = nc.dram_tensor("wkT", [H, DH], BF16, kind="ExternalInput").ap()
    wvT = nc.dram_tensor("wvT", [H, DH], BF16, kind="ExternalInput").ap()
    woT = nc.dram_tensor("woT", [DH, H], BF16, kind="ExternalInput").ap()
    bq = nc.dram_tensor("bq", [DH], F32, kind="ExternalInput").ap()
    bk = nc.dram_tensor("bk", [DH], F32, kind="ExternalInput").ap()
    bv = nc.dram_tensor("bv", [DH], F32, kind="ExternalInput").ap()
    cmat = nc.dram_tensor("cmat", [P, P], F32, kind="ExternalInput").ap()
    out = nc.dram_tensor("out", [S, H], F32, kind="ExternalOutput").ap()

    with (
        TileContext(nc) as tc,
        ExitStack() as ctx,
        nc.allow_low_precision(reason="bf16 attention; float32r elsewhere"),
    ):
        consts = ctx.enter_context(tc.tile_pool(name="consts", bufs=1))
        persist = ctx.enter_context(tc.tile_pool(name="persist", bufs=1))
        wpool = ctx.enter_context(tc.tile_pool(name="w", bufs=1))
        xpool = ctx.enter_context(tc.tile_pool(name="x", bufs=2))
        ptp = ctx.enter_context(tc.tile_pool(name="pt", bufs=PT_BUFS))
        pstp = ctx.enter_context(tc.tile_pool(name="pst", bufs=12))
        rcp = ctx.enter_context(tc.tile_pool(name="rc", bufs=3))
        stg = ctx.enter_context(tc.tile_pool(name="p3s", bufs=4))
        bcp = ctx.enter_context(tc.tile_pool(name="bc", bufs=2))
        ps_sc = ctx.enter_context(tc.tile_pool(name="ps_sc", bufs=3, space="PSUM"))
        ps_pv = ctx.enter_context(tc.tile_pool(name="ps_pv", bufs=2, space="PSUM"))

        # ---- constants ---------------------------------------------------
        # cmat rows: 0 = head-A indicator, 32 = head-B indicator, 64 = ones
        ind_sb = consts.tile([P, P], F32R)
        nc.sync.dma_start(ind_sb[:], cmat.bitcast(F32R))
        bq_sb = consts.tile([P, NDQ], F32)
        nc.sync.dma_start(bq_sb[:], bq.rearrange("(o p) -> p o", p=P))
        bk_sb = consts.tile([P, NDQ], F32)
        nc.sync.dma_start(bk_sb[:], bk.rearrange("(o p) -> p o", p=P))
        bv_sb = consts.tile([P, DH], F32R)
        nc.sync.dma_start(bv_sb[64:65, :], bv[None, :].bitcast(F32R))

        # Band masks, shared over the head dim: mask[c][p, h, f] = f >= 128c + p
        mask_sb = consts.tile([P, NHP, 2, SQC], BF16)
        nc.any.memset(mask_sb, 1.0)
        for c in range(4):
            nc.gpsimd.affine_select(
                mask_sb[:, c, :, :], mask_sb[:, c, :, :],
                pattern=[[0, 2], [1, SQC]],
                compare_op=mybir.AluOpType.is_ge,
                fill=0.0,
                base=-P * c,
                channel_multiplier=-1,
            )

        # ---- persistent tensors ------------------------------------------
        v_sb = persist.tile([P, NSK, DH], BF16)      # V (s, dv)
        qt_sb = persist.tile([P, NDQ, S], BF16)      # Q^T (dq, s)
        kt_sb = persist.tile([P, NDQ, S], BF16)      # K^T (dq, s)
        outT_sb = persist.tile([P, NDQ, S], BF16)    # normalized attn out^T
        # [P, 2*ko + mcol, :] == flat [P, NDQ, H] slice [:, ko, mcol*512:...]
        wo_bf = persist.tile([P, HO, SQC], BF16)
        bvb_sb = persist.tile([P, DH], F32R)         # bv broadcast to 128 rows

        # ---- weights ------------------------------------------------------
        # x chunk 0 first (finely split so the first Q matmul can start as
        # soon as x[o=0] and wq[o=0] land), then the weights.
        w_sb = {}
        xq_tiles = [None] * NSQ
        xq_tiles[0] = xpool.tile([P, HO, SQC], BF16, name="xq")
        xr = xT.rearrange("(o p) s -> p o s", p=P)
        for o in range(HO):
            nc.sync.dma_start(xq_tiles[0][:, o, :], xr[:, o, 0:SQC])
        for name, wT in (("q", wqT), ("k", wkT), ("v", wvT)):
            w_sb[name] = wpool.tile([P, HO, DH], BF16, name=f"w{name}_sb")
            for o in range(HO):
                nc.scalar.dma_start(
                    w_sb[name][:, o, :],
                    wT.rearrange("(o p) d -> p o d", p=P)[:, o, :],
                )
        for ko in range(NDQ):
            for m2 in range(2):
                nc.scalar.dma_start(
                    wo_bf[:, 2 * ko + m2, :],
                    woT.rearrange("(o p) m -> p o m", p=P)[
                        :, ko, m2 * SQC : (m2 + 1) * SQC
                    ],
                )

        # bv broadcast tile: ones-row x bv row
        ps = ps_sc.tile([P, 2, SQC], F32, name="sc_ps")
        nc.tensor.matmul(
            ps[:, 0, :], ind_sb[64:65, :], bv_sb[64:65, :],
            start=True, stop=True, tile_position=(64, 0),
        )
        nc.vector.tensor_copy(bvb_sb[:], ps[:, 0, :])

        # pre-zero the pt ring so suffix-exp prefixes are finite
        for _ in range(PT_BUFS):
            ptz = ptp.tile([P, 2, SQC], BF16, name="pt")
            nc.vector.memset(ptz[:], 0.0)

        def emit_p1(sc):
            ss = slice(sc * SQC, (sc + 1) * SQC)
            xq = xq_tiles[sc]
            # q/k interleaved per t-tile so attention block hp=t can start
            # as soon as its pair of drains lands
            for t in range(NDQ):
                pps = ps_sc.tile([P, 2, SQC], F32, name="sc_ps")
                for td, (name, bias_sb, scale, tgt) in enumerate((
                    ("q", bq_sb, SCALE, qt_sb),
                    ("k", bk_sb, 1.0, kt_sb),
                )):
                    for o in range(HO):
                        nc.tensor.matmul(
                            pps[:, td, :],
                            w_sb[name][:, o, t * P : (t + 1) * P],
                            xq[:, o, :],
                            start=(o == 0), stop=(o == HO - 1),
                        )
                    nc.vector.tensor_scalar(
                        tgt[:, t, ss], pps[:, td, :],
                        scale, bias_sb[:, t : t + 1],
                        op0=mybir.AluOpType.mult, op1=mybir.AluOpType.add,
                    )
            for vp in range(2):
                pps = ps_sc.tile([P, 2, SQC], F32, name="sc_ps")
                for vd in range(2):
                    stq = 2 * vp + vd
                    st_i = sc * (SQC // P) + stq
                    for o in range(HO):
                        nc.tensor.matmul(
                            pps[:, vd, :],
                            xq[:, o, stq * P : (stq + 1) * P],
                            w_sb["v"][:, o, :],
                            start=(o == 0), stop=(o == HO - 1),
                        )
                    nc.vector.tensor_add(v_sb[:, st_i, :], pps[:, vd, :], bvb_sb[:])

        def emit_block(hp, i):
            """Attention for head-pair hp, q-chunk i (512 rows)."""
            nj, nm = 4 * i + 4, 2 * i + 2
            sq = slice(i * SQC, (i + 1) * SQC)
            pv_ps = ps_pv.tile([P, SQC], F32, name="pv_ps")

            def emit_pv(ent):
                pts, m = ent
                for d in range(2):
                    j = 2 * m + d
                    for h in range(2):
                        dv = slice(hp * P + h * 64, hp * P + h * 64 + 64)
                        nc.tensor.matmul(
                            pv_ps[h * 64 : h * 64 + 64, :],
                            v_sb[:, j, dv],
                            pts[d][:, h, :],
                            start=(j == 0), stop=(j == nj - 1),
                            tile_position=(0, h * 64),
                        )

            pend = deque()
            # per-head pairwise reduction trees over the pair sums; each
            # level is a bf16 DVE add, the final [128,512] tile feeds one
            # ones-matmul per head for the partition sum.
            trees = [[], []]

            def tree_push(h, tile, lvl=0):
                tl = trees[h]
                while len(tl) <= lvl:
                    tl.append(None)
                if tl[lvl] is None:
                    tl[lvl] = tile
                    return
                acc = pstp.tile([P, SQC], BF16, name="pst")
                nc.vector.tensor_add(acc[:], tl[lvl][:], tile[:])
                tl[lvl] = None
                tree_push(h, acc, lvl + 1)

            def tree_finish(h):
                acc = None
                for tile in trees[h]:
                    if tile is None:
                        continue
                    if acc is None:
                        acc = tile
                        continue
                    nxt = pstp.tile([P, SQC], BF16, name="pst")
                    nc.vector.tensor_add(nxt[:], acc[:], tile[:])
                    acc = nxt
                return acc

            for m in range(nm):
                pts = []
                for d in range(2):
                    j = 2 * m + d
                    sk = slice(j * P, (j + 1) * P)
                    sc_ps = ps_sc.tile([P, 2, SQC], F32, name="sc_ps")
                    for h in range(2):
                        pb = 64 * h
                        nc.tensor.matmul(
                            sc_ps[:, h, :],
                            kt_sb[pb : pb + 64, hp, sk],
                            qt_sb[pb : pb + 64, hp, sq],
                            start=True, stop=True,
                            tile_position=(pb, 0),
                        )
                    pt = ptp.tile([P, 2, SQC], BF16, name="pt")
                    c = j - 4 * i
                    off = max(0, P * c)
                    nc.scalar.activation(
                        pt[:, :, off:], sc_ps[:, :, off:],
                        mybir.ActivationFunctionType.Exp,
                    )
                    if c >= 0:  # diagonal chunk: causal band mask
                        # full-width: zeroes the unwritten prefix too
                        nc.vector.tensor_mul(pt[:], pt[:], mask_sb[:, c, :, :])
                    pts.append(pt)
                for h in range(2):
                    pst_t = pstp.tile([P, SQC], BF16, name="pst")
                    nc.vector.tensor_add(
                        pst_t[:], pts[0][:, h, :], pts[1][:, h, :]
                    )
                    tree_push(h, pst_t)
                pend.append((pts, m))
                if len(pend) >= 2:
                    emit_pv(pend.popleft())
            while pend:
                emit_pv(pend.popleft())

            bc_sb = bcp.tile([P, SQC], F32R, name="bc_sb")
            for h, rows in ((0, slice(0, 64)), (1, slice(64, 128))):
                dsum = tree_finish(h)
                dar = rcp.tile([P, SQC], F32, name="dar")
                nc.gpsimd.partition_all_reduce(
                    dar[:], dsum[:], channels=P,
                    reduce_op=bass_isa.ReduceOp.add,
                )
                nc.vector.reciprocal(bc_sb[rows, :], dar[rows, :])
            nc.vector.tensor_mul(outT_sb[:, hp, sq], pv_ps[:], bc_sb[:])

        def emit_p3(sc, ko_major=False):
            if not ko_major:
                for st_i in range(SQC // P):
                    s0 = sc * SQC + st_i * P
                    ss = slice(s0, s0 + P)
                    pps = ps_sc.tile([P, 2, SQC], F32, name="sc_ps")
                    for mcol in range(2):
                        for ko in range(NDQ):
                            nc.tensor.matmul(
                                pps[:, mcol, :],
                                outT_sb[:, ko, ss],
                                wo_bf[:, 2 * ko + mcol, :],
                                start=(ko == 0), stop=(ko == NDQ - 1),
                            )
                    for mcol in range(2):
                        ms = slice(mcol * SQC, (mcol + 1) * SQC)
                        ot = stg.tile([P, SQC], F32, name="o_stage")
                        nc.scalar.copy(ot[:], pps[:, mcol, :])
                        nc.sync.dma_start(out[ss, ms], ot[:])
                return
            # ko-major: all 8 accumulators live at once so the ko<3 matmuls
            # overlap the final block's normalization drain (last chunk only)
            accs = []
            for st_i in range(3):
                t = ps_sc.tile([P, 2, SQC], F32, name="sc_ps")
                accs.append((t[:, 0, :], t[:, 1, :], t))
            t0 = ps_pv.tile([P, SQC], F32, name="pv_ps")
            t1 = ps_pv.tile([P, SQC], F32, name="pv_ps")
            accs.append((t0[:], t1[:], None))
            for ko in range(NDQ):
                for st_i in range(SQC // P):
                    for mcol in range(2):
                        nc.tensor.matmul(
                            accs[st_i][mcol],
                            outT_sb[:, ko, slice(sc * SQC + st_i * P,
                                                 sc * SQC + st_i * P + P)],
                            wo_bf[:, 2 * ko + mcol, :],
                            start=(ko == 0), stop=(ko == NDQ - 1),
                        )
            for st_i in range(SQC // P):
                s0 = sc * SQC + st_i * P
                ss = slice(s0, s0 + P)
                for mcol in range(2):
                    ms = slice(mcol * SQC, (mcol + 1) * SQC)
                    ot = stg.tile([P, SQC], F32, name="o_stage")
                    nc.scalar.copy(ot[:], accs[st_i][mcol])
                    nc.sync.dma_start(out[ss, ms], ot[:])

        for sc in range(NSQ):
            if "1" in phases:
                emit_p1(sc)
            if sc + 1 < NSQ:  # prefetch next x chunk; overlaps with P2/P3
                xq_tiles[sc + 1] = xpool.tile([P, HO, SQC], BF16, name="xq")
                nc.sync.dma_start(
                    xq_tiles[sc + 1][:],
                    xr[:, :, (sc + 1) * SQC : (sc + 2) * SQC],
                )
            if sc > 0 and "3" in phases:
                emit_p3(sc - 1)
            if "2" in phases:
                for hp in range(NHP):
                    emit_block(hp, sc)
        if "3" in phases:
            emit_p3(NSQ - 1, ko_major=True)

    nc.compile()
    return nc


_NC_CACHE = [None]
LAST_RESULT = [None]


def make_in_maps(inputs):
    """Per-core input maps from the full-problem input dict."""
    x, Wq, bq, Wk, bk, Wv, bv, Wo = (
        np.asarray(inputs[k], dtype=np.float32)
        for k in ("x", "Wq", "bq", "Wk", "bk", "Wv", "bv", "Wo")
    )
    cmat = np.zeros((P, P), np.float32)
    cmat[0, 0:64] = 1.0    # head-A indicator
    cmat[32, 64:128] = 1.0  # head-B indicator
    cmat[64, :] = 1.0       # ones row (bias broadcast)
    in_maps = []
    for c in range(8):
        b, g = c // 2, c % 2
        hs = slice(DH * g, DH * (g + 1))
        bf16 = ml_dtypes.bfloat16
        in_maps.append({
            "xT": np.ascontiguousarray(x[b].T).astype(bf16),
            "wqT": np.ascontiguousarray(Wq[hs].T).astype(bf16),
            "wkT": np.ascontiguousarray(Wk[hs].T).astype(bf16),
            "wvT": np.ascontiguousarray(Wv[hs].T).astype(bf16),
            "woT": np.ascontiguousarray(Wo[:, hs].T).astype(bf16),
            "bq": np.ascontiguousarray(bq[hs]) * np.float32(SCALE),
            "bk": np.ascontiguousarray(bk[hs]),
            "bv": np.ascontiguousarray(bv[hs]),
            "cmat": cmat,
        })
    return in_maps


def kernel(x, Wq, bq, Wk, bk, Wv, bv, Wo, bo):
    if _NC_CACHE[0] is None:
        _NC_CACHE[0] = build_kernel()
    nc = _NC_CACHE[0]

    in_maps = make_in_maps(dict(
        x=x, Wq=Wq, bq=bq, Wk=Wk, bk=bk, Wv=Wv, bv=bv, Wo=Wo,
    ))
    trace = bool(os.environ.get("BASS_PROFILE"))
    res = run_bass_kernel_spmd(
        nc, in_maps, core_ids=list(range(8)), trace=trace,
        tmpdir=os.environ.get("BASS_PROFILE_DIR") or None,
    )
    LAST_RESULT[0] = res
    bo = np.asarray(bo, dtype=np.float32)
    out = np.empty((B, S, H), np.float32)
    for b in range(B):
        out[b] = res.results[2 * b]["out"] + res.results[2 * b + 1]["out"] + bo
    return out
